# revision 27
# baseline (speedup 1.0000x reference)
"""Trainium2 Bass kernel for the hard-negative-mining set loss (v6).

Structure (all HW-probe-proven primitives):
  * host precomputes pos_idx (pure target metadata); phase B eliminated.
  * mining (per core, 1024 local rows): exp pass -> rsum; DVE computes
    bf16 mining values -probs - onehot; PE-transposes (bf16) to
    [class, row]; DVE max8 + first-index per class into 3D top8 tiles;
    8KB AllGather of per-class (vmax, enc) tables.
  * combine -> per-class neg row table in DRAM -> ONE batched
    single-element indirect gather by target -> per-anchor neg index.
  * xp/xn rows: plain per-tile indirect gathers + DVE adds; one fused
    target-logit extraction per tile on the summed logits.
HW notes (probed): tensor_tensor_reduce, batched multi-row gathers,
batched-offset + compute_op=add chains all hang/corrupt on HW; batched
single-element gathers and the DRAM-table double hop work.
"""

import numpy as np

import concourse.bass as bass
import concourse.bacc as bacc
import concourse.tile as tile
from concourse import mybir
from concourse.bass_utils import run_bass_kernel_spmd
from concourse.tile import add_dep_helper

B, C = 8192, 1024
NCORES = 8
BL = B // NCORES      # 1024 local rows per core
NT = BL // 128        # 8 row tiles
CT = C // 128         # 8 class tiles
BIGI = 16384.0        # index encoding base: enc = BIGI - global_row_idx
SHIFT_A = 10.0        # softmax shift (x ~ N(0,1): rowmax << SHIFT_A)
SHIFT_C = 14.0        # summed-logits shift (3 logits per entry)
F32 = mybir.dt.float32
BF16 = mybir.dt.bfloat16
I32 = mybir.dt.int32
U32 = mybir.dt.uint32
AX = mybir.AxisListType
OP = mybir.AluOpType
AF = mybir.ActivationFunctionType


def build_nc():
    nc = bacc.Bacc("TRN2", target_bir_lowering=False, debug=False,
                   num_devices=NCORES)

    x_d = nc.dram_tensor("x", [B, C], F32, kind="ExternalInput")
    xloc_d = nc.dram_tensor("xloc", [BL, C], F32, kind="ExternalInput")
    cidb_d = nc.dram_tensor("cidb", [128, C], F32, kind="ExternalInput")
    tcols_d = nc.dram_tensor("tcols", [128, NT], F32, kind="ExternalInput")
    posloc_d = nc.dram_tensor("posloc", [128, NT], I32, kind="ExternalInput")
    tgtloc_d = nc.dram_tensor("tgtloc", [128, NT], I32, kind="ExternalInput")
    bigoff_d = nc.dram_tensor("bigoff", [128, 1], F32, kind="ExternalInput")
    identb_d = nc.dram_tensor("identb", [128, 128], BF16, kind="ExternalInput")
    out_d = nc.dram_tensor("partial", [1, 1], F32, kind="ExternalOutput")

    # collective bounce: col ct = vmax of class ct*128+p, col CT+ct = enc
    cc_in = nc.dram_tensor("cc_in", [128, 2 * CT], F32)
    cc_out = nc.dram_tensor("cc_out", [NCORES, 128, 2 * CT], F32)
    negtab_d = nc.dram_tensor("negtab", [C, 1], I32)

    with tile.TileContext(nc) as tc:
        with (
            tc.tile_pool(name="persist", bufs=1) as pp,
            tc.tile_pool(name="dumppool", bufs=2) as dp,
            tc.tile_pool(name="nvpool", bufs=2) as nvp,
            tc.tile_pool(name="small", bufs=4) as smp,
            tc.tile_pool(name="gxp", bufs=3) as gxp,
            tc.tile_pool(name="gxn", bufs=3) as gxn,
            tc.tile_pool(name="psA", bufs=1, space="PSUM") as psa,
            tc.tile_pool(name="psB", bufs=2, space="PSUM") as psb,
        ):
            # ---------- input loads ----------
            cidb = pp.tile([128, C], F32, tag="cidb")
            nc.scalar.dma_start(out=cidb, in_=cidb_d.ap())
            tcols = pp.tile([128, NT], F32, tag="tcols")
            nc.scalar.dma_start(out=tcols, in_=tcols_d.ap())
            posloc = pp.tile([128, NT], I32, tag="posloc")
            nc.scalar.dma_start(out=posloc, in_=posloc_d.ap())
            tgtloc = pp.tile([128, NT], I32, tag="tgtloc")
            nc.scalar.dma_start(out=tgtloc, in_=tgtloc_d.ap())
            bigoff = pp.tile([128, 1], F32, tag="bigoff")
            nc.scalar.dma_start(out=bigoff, in_=bigoff_d.ap())
            identb = pp.tile([128, 128], BF16, tag="identb")
            nc.scalar.dma_start(out=identb, in_=identb_d.ap())

            # xloc: 8 tiles, round-robin across the two hwdge issue queues
            dma_engs = [nc.sync, nc.scalar]
            xloc = []
            for t in range(NT):
                xt = pp.tile([128, C], F32, tag=f"xloc{t}")
                dma_engs[t % 2].dma_start(
                    out=xt, in_=xloc_d.ap()[t * 128:(t + 1) * 128, :])
                xloc.append(xt)

            ones = pp.tile([128, 1], F32, tag="ones")
            nc.vector.memset(ones, 1.0)
            shA = pp.tile([128, 1], F32, tag="shA")
            nc.vector.memset(shA, -SHIFT_A)
            shC = pp.tile([128, 1], F32, tag="shC")
            nc.vector.memset(shC, -SHIFT_C)

            # ---------- phase A: hardest-negative mining (exp domain, bf16) --
            # onehot(target) on gpsimd (frees DVE, the mining bottleneck)
            eqm = []
            for t in range(NT):
                eq = pp.tile([128, C], F32, tag=f"eqm{t}")
                nc.gpsimd.tensor_scalar(out=eq, in0=cidb,
                                        scalar1=tcols[:, t:t + 1], scalar2=None,
                                        op0=OP.is_equal)
                eqm.append(eq)

            negrr = pp.tile([128, NT], F32, tag="negrr")
            hb = []       # mining values, bf16: -probs - onehot
            exp_ins = []
            for t in range(NT):
                dump = dp.tile([128, C], F32, tag="dump")
                rsum = smp.tile([128, 1], F32, tag="rsum")
                e = nc.scalar.activation(out=dump, in_=xloc[t], func=AF.Exp,
                                         bias=shA, scale=1.0, accum_out=rsum)
                exp_ins.append(e)
                nc.vector.reciprocal(out=negrr[:, t:t + 1], in_=rsum)
                nc.vector.tensor_scalar(out=negrr[:, t:t + 1],
                                        in0=negrr[:, t:t + 1], scalar1=-1.0,
                                        scalar2=None, op0=OP.mult)
                ht = pp.tile([128, C], BF16, tag=f"hb{t}")
                nc.vector.scalar_tensor_tensor(out=ht, in0=dump,
                                               scalar=negrr[:, t:t + 1],
                                               in1=eqm[t], op0=OP.mult,
                                               op1=OP.subtract)
                hb.append(ht)

            # xp gather (host-precomputed pos indices): per-tile issues.
            # Dep-pinned after tile 3's exp so the gather DMA does not
            # steal HBM bandwidth from the xloc load.
            xpg = []
            for t in range(NT):
                xpt = gxp.tile([128, C], F32, tag="xp")
                gi = nc.gpsimd.indirect_dma_start(
                    out=xpt, out_offset=None,
                    in_=x_d.ap(),
                    in_offset=bass.IndirectOffsetOnAxis(
                        ap=posloc[:, t:t + 1], axis=0))
                add_dep_helper(gi.ins, exp_ins[0].ins, sync=False)
                xpg.append(xpt)

            # transpose to [class, row]; top-1 + index per class
            ccall = pp.tile([128, 2 * CT], F32, tag="ccall")
            idxcat = pp.tile([128, CT], F32, tag="idxcat")
            for g in range(4):
                psts = []
                for ci in range(2):
                    pst = psa.tile([128, C], BF16, tag=f"pst{ci}")
                    psts.append(pst)
                for t in range(NT):
                    for ci in range(2):
                        ct = g * 2 + ci
                        nc.tensor.transpose(
                            out=psts[ci][:, t * 128:(t + 1) * 128],
                            in_=hb[t][:, ct * 128:(ct + 1) * 128],
                            identity=identb)
                for ci in range(2):
                    ct = g * 2 + ci
                    nvT = nvp.tile([128, C], BF16, tag="nvT")
                    nc.scalar.copy(out=nvT, in_=psts[ci])
                    top8v = smp.tile([128, 8], BF16, tag="top8v")
                    nc.vector.max(out=top8v, in_=nvT)
                    top8i = smp.tile([128, 8], U32, tag="top8i")
                    nc.vector.max_index(out=top8i, in_max=top8v, in_values=nvT)
                    nc.vector.tensor_copy(out=idxcat[:, ct:ct + 1],
                                          in_=top8i[:, 0:1])
                    nc.vector.tensor_copy(out=ccall[:, ct:ct + 1],
                                          in_=top8v[:, 0:1])
            # enc = (BIGI - core_off) - idx, all classes in one op
            nc.vector.tensor_scalar(out=ccall[:, CT:2 * CT], in0=idxcat,
                                    scalar1=bigoff, scalar2=-1.0,
                                    op0=OP.subtract, op1=OP.mult)
            nc.sync.dma_start(out=cc_in.ap(), in_=ccall)

            # ---------- AllGather ----------
            nc.gpsimd.collective_compute(
                "AllGather", OP.bypass,
                replica_groups=[list(range(NCORES))],
                ins=[cc_in.ap().opt()], outs=[cc_out.ap().opt()])

            g8 = pp.tile([128, NCORES, 2 * CT], F32, tag="g8")
            gsrc = bass.AP(tensor=cc_out.ap().tensor, offset=0,
                           ap=[[2 * CT, 128], [128 * 2 * CT, NCORES],
                               [1, 2 * CT]])
            nc.scalar.dma_start(out=g8, in_=gsrc)

            # ---- overlapped with the collective: fold xp rows into xloc ----
            for t in range(NT):
                nc.vector.tensor_tensor(out=xloc[t], in0=xloc[t],
                                        in1=xpg[t], op=OP.add)

            # ---------- global combine: per-class hardest negative ----------
            negidxf = pp.tile([128, CT], F32, tag="negidxf")
            for ct in range(CT):
                gv = smp.tile([128, 1], F32, tag="gv")
                nc.vector.tensor_reduce(out=gv, in_=g8[:, :, ct], axis=AX.X,
                                        op=OP.max)
                mm = smp.tile([128, NCORES], F32, tag="mm")
                nc.vector.tensor_tensor(out=mm, in0=g8[:, :, ct],
                                        in1=gv.to_broadcast([128, NCORES]),
                                        op=OP.is_ge)
                cand = smp.tile([128, NCORES], F32, tag="cand")
                nc.vector.tensor_tensor(out=cand, in0=mm,
                                        in1=g8[:, :, CT + ct], op=OP.mult)
                genc = smp.tile([128, 1], F32, tag="genc")
                nc.vector.tensor_reduce(out=genc, in_=cand, axis=AX.X,
                                        op=OP.max)
                nc.vector.tensor_scalar(out=negidxf[:, ct:ct + 1], in0=genc,
                                        scalar1=-1.0, scalar2=BIGI,
                                        op0=OP.mult, op1=OP.add)
            negtabi = pp.tile([128, CT], I32, tag="negtabi")
            nc.vector.tensor_copy(out=negtabi, in_=negidxf)
            ntdst = bass.AP(tensor=negtab_d.ap().tensor, offset=0,
                            ap=[[1, 128], [128, CT]])
            nc.sync.dma_start(out=ntdst, in_=negtabi)

            # ---------- phase C: per-anchor neg (double hop) + CE ----------
            ext = pp.tile([128, NT], F32, tag="ext")
            rsumC = pp.tile([128, NT], F32, tag="rsumC")
            lnrC = pp.tile([128, NT], F32, tag="lnrC")
            expC_ins = []
            for t in range(NT):
                negoff = smp.tile([128, 1], I32, tag="negoff")
                nc.gpsimd.indirect_dma_start(
                    out=negoff, out_offset=None,
                    in_=negtab_d.ap(),
                    in_offset=bass.IndirectOffsetOnAxis(
                        ap=tgtloc[:, t:t + 1], axis=0))
                xnt = gxn.tile([128, C], F32, tag="xn")
                nc.gpsimd.indirect_dma_start(
                    out=xnt, out_offset=None,
                    in_=x_d.ap(),
                    in_offset=bass.IndirectOffsetOnAxis(ap=negoff, axis=0))
                nc.vector.tensor_tensor(out=xloc[t], in0=xloc[t], in1=xnt,
                                        op=OP.add)
                dumpE = dp.tile([128, C], F32, tag="dump")
                nc.vector.tensor_tensor(out=dumpE, in0=xloc[t], in1=eqm[t],
                                        op=OP.mult)
                nc.vector.tensor_reduce(out=ext[:, t:t + 1], in_=dumpE,
                                        axis=AX.X, op=OP.add)
                dump = dp.tile([128, C], F32, tag="dump")
                rsc = smp.tile([128, 1], F32, tag="rsc")
                e = nc.scalar.activation(out=dump, in_=xloc[t], func=AF.Exp,
                                         bias=shC, scale=1.0, accum_out=rsc)
                nc.vector.tensor_copy(out=rsumC[:, t:t + 1], in_=rsc)
                expC_ins.append(e)
            ln = nc.scalar.activation(out=lnrC, in_=rsumC, func=AF.Ln)
            add_dep_helper(ln.ins, expC_ins[-1].ins, sync=False)

            # loss partial: sum_t [ln(rsum) + SHIFT_C - tval]
            li = pp.tile([128, NT], F32, tag="li")
            nc.vector.tensor_scalar(out=li, in0=lnrC, scalar1=SHIFT_C,
                                    scalar2=None, op0=OP.add)
            nc.vector.tensor_tensor(out=li, in0=li, in1=ext, op=OP.subtract)
            acc = pp.tile([128, 1], F32, tag="acc")
            nc.vector.tensor_reduce(out=acc, in_=li, axis=AX.X, op=OP.add)

            pss = psb.tile([1, 1], F32, tag="psum_out")
            nc.tensor.matmul(pss, lhsT=acc, rhs=ones, start=True, stop=True)
            outt = smp.tile([1, 1], F32, tag="outt")
            nc.vector.tensor_copy(out=outt, in_=pss)
            nc.sync.dma_start(out=out_d.ap(), in_=outt)

    nc.compile()
    return nc


_NC_CACHE = {}


def get_nc():
    if "nc" not in _NC_CACHE:
        _NC_CACHE["nc"] = build_nc()
    return _NC_CACHE["nc"]


def _pos_idx_host(tgt):
    """pos_idx[i] = first same-label index != i (argmax semantics of the
    reference: first j with target[j]==target[i], j != i)."""
    order = np.argsort(tgt, kind="stable")
    pos = np.zeros(B, dtype=np.int32)
    srt = tgt[order]
    starts = np.searchsorted(srt, np.arange(srt[-1] + 1), side="left")
    for c in range(len(starts)):
        lo = starts[c]
        hi = starts[c + 1] if c + 1 < len(starts) else B
        idxs = order[lo:hi]
        if len(idxs) == 0:
            continue
        mn = idxs.min()
        pos[idxs] = mn
        if len(idxs) > 1:
            second = np.partition(idxs, 1)[1]
            pos[mn] = second
        else:
            pos[mn] = 0
    return pos


def make_in_maps(x, target):
    x = np.ascontiguousarray(np.asarray(x, dtype=np.float32))
    tgt = np.asarray(target).astype(np.int64)
    assert x.shape == (B, C) and tgt.shape == (B,)

    import ml_dtypes
    cid = np.arange(C, dtype=np.float32)
    cidb_full = np.ascontiguousarray(np.broadcast_to(cid, (128, C)))
    identb = np.eye(128, dtype=ml_dtypes.bfloat16)

    pos = _pos_idx_host(tgt)

    in_maps = []
    for k in range(NCORES):
        rows = slice(k * BL, (k + 1) * BL)
        tl = tgt[rows].astype(np.float32)
        in_maps.append({
            "x": x,
            "xloc": np.ascontiguousarray(x[rows]),
            "cidb": cidb_full,
            "tcols": np.ascontiguousarray(tl.reshape(NT, 128).T),
            "posloc": np.ascontiguousarray(
                pos[rows].reshape(NT, 128).T.astype(np.int32)),
            "tgtloc": np.ascontiguousarray(
                tgt[rows].reshape(NT, 128).T.astype(np.int32)),
            "bigoff": np.full((128, 1), BIGI - k * BL, dtype=np.float32),
            "identb": identb,
        })
    return in_maps


def kernel(x, target):
    nc = get_nc()
    in_maps = make_in_maps(x, target)
    res = run_bass_kernel_spmd(nc, in_maps, core_ids=list(range(NCORES)))
    total = sum(float(res.results[k]["partial"][0, 0]) for k in range(NCORES))
    return np.float32(total / B)


# revision 29
# speedup vs baseline: 1.6622x; 1.6622x over previous
"""Trainium2 Bass kernel for the hard-negative-mining set loss (v6).

Structure (all HW-probe-proven primitives):
  * host precomputes pos_idx (pure target metadata); phase B eliminated.
  * mining (per core, 1024 local rows): exp pass -> rsum; DVE computes
    bf16 mining values -probs - onehot; PE-transposes (bf16) to
    [class, row]; DVE max8 + first-index per class into 3D top8 tiles;
    8KB AllGather of per-class (vmax, enc) tables.
  * combine -> per-class neg row table in DRAM -> ONE batched
    single-element indirect gather by target -> per-anchor neg index.
  * xp/xn rows: plain per-tile indirect gathers + DVE adds; one fused
    target-logit extraction per tile on the summed logits.
HW notes (probed): tensor_tensor_reduce, batched multi-row gathers,
batched-offset + compute_op=add chains all hang/corrupt on HW; batched
single-element gathers and the DRAM-table double hop work.
"""

import numpy as np

import concourse.bass as bass
import concourse.bacc as bacc
import concourse.tile as tile
from concourse import mybir
from concourse.bass_utils import run_bass_kernel_spmd
from concourse.tile import add_dep_helper

B, C = 8192, 1024
NCORES = 8
BL = B // NCORES      # 1024 local rows per core
NT = BL // 128        # 8 row tiles
CT = C // 128         # 8 class tiles
BIGI = 16384.0        # index encoding base: enc = BIGI - global_row_idx
SHIFT_A = 10.0        # softmax shift (x ~ N(0,1): rowmax << SHIFT_A)
SHIFT_C = 14.0        # summed-logits shift (3 logits per entry)
F32 = mybir.dt.float32
BF16 = mybir.dt.bfloat16
I32 = mybir.dt.int32
U32 = mybir.dt.uint32
AX = mybir.AxisListType
OP = mybir.AluOpType
AF = mybir.ActivationFunctionType


def build_nc():
    nc = bacc.Bacc("TRN2", target_bir_lowering=False, debug=False,
                   num_devices=NCORES)

    x_d = nc.dram_tensor("x", [B, C], F32, kind="ExternalInput")
    xloc_d = nc.dram_tensor("xloc", [BL, C], F32, kind="ExternalInput")
    cidb_d = nc.dram_tensor("cidb", [128, C], F32, kind="ExternalInput")
    tcols_d = nc.dram_tensor("tcols", [128, NT], F32, kind="ExternalInput")
    posloc_d = nc.dram_tensor("posloc", [128, NT], I32, kind="ExternalInput")
    tgtloc_d = nc.dram_tensor("tgtloc", [128, NT], I32, kind="ExternalInput")
    bigoff_d = nc.dram_tensor("bigoff", [128, 1], F32, kind="ExternalInput")
    identb_d = nc.dram_tensor("identb", [128, 128], BF16, kind="ExternalInput")
    out_d = nc.dram_tensor("partial", [1, 1], F32, kind="ExternalOutput")

    # collective bounce: col ct = vmax of class ct*128+p, col CT+ct = enc
    cc_in = nc.dram_tensor("cc_in", [128, 2 * CT], F32)
    cc_out = nc.dram_tensor("cc_out", [NCORES, 128, 2 * CT], F32)
    negtab_d = nc.dram_tensor("negtab", [C, 1], I32)

    with tile.TileContext(nc) as tc:
        with (
            tc.tile_pool(name="persist", bufs=1) as pp,
            tc.tile_pool(name="dumppool", bufs=2) as dp,
            tc.tile_pool(name="nvpool", bufs=2) as nvp,
            tc.tile_pool(name="small", bufs=4) as smp,
            tc.tile_pool(name="gxp", bufs=4) as gxp,
            tc.tile_pool(name="gxn", bufs=4) as gxn,
            tc.tile_pool(name="psA", bufs=2, space="PSUM") as psa,
            tc.tile_pool(name="psB", bufs=2, space="PSUM") as psb,
        ):
            # ---------- input loads ----------
            cidb = pp.tile([128, C], F32, tag="cidb")
            nc.scalar.dma_start(out=cidb, in_=cidb_d.ap())
            tcols = pp.tile([128, NT], F32, tag="tcols")
            nc.scalar.dma_start(out=tcols, in_=tcols_d.ap())
            posloc = pp.tile([128, NT], I32, tag="posloc")
            nc.scalar.dma_start(out=posloc, in_=posloc_d.ap())
            tgtloc = pp.tile([128, NT], I32, tag="tgtloc")
            nc.scalar.dma_start(out=tgtloc, in_=tgtloc_d.ap())
            bigoff = pp.tile([128, 1], F32, tag="bigoff")
            nc.scalar.dma_start(out=bigoff, in_=bigoff_d.ap())
            identb = pp.tile([128, 128], BF16, tag="identb")
            nc.scalar.dma_start(out=identb, in_=identb_d.ap())

            # xloc: 8 tiles, round-robin across the two hwdge issue queues
            dma_engs = [nc.sync, nc.scalar]
            xloc = []
            for t in range(NT):
                xt = pp.tile([128, C], F32, tag=f"xloc{t}")
                dma_engs[t % 2].dma_start(
                    out=xt, in_=xloc_d.ap()[t * 128:(t + 1) * 128, :])
                xloc.append(xt)

            ones = pp.tile([128, 1], F32, tag="ones")
            nc.vector.memset(ones, 1.0)
            shA = pp.tile([128, 1], F32, tag="shA")
            nc.vector.memset(shA, -SHIFT_A)
            shC = pp.tile([128, 1], F32, tag="shC")
            nc.vector.memset(shC, -SHIFT_C)

            # ---------- phase A: hardest-negative mining (exp domain, bf16) --
            eqm = []      # onehot(target), f32: mining exclusion + extraction
            for t in range(NT):
                eq = pp.tile([128, C], F32, tag=f"eqm{t}")
                nc.vector.tensor_scalar(out=eq, in0=cidb,
                                        scalar1=tcols[:, t:t + 1], scalar2=None,
                                        op0=OP.is_equal)
                eqm.append(eq)

            negrr = pp.tile([128, NT], F32, tag="negrr")
            hb = []       # mining values, bf16: -probs - onehot
            exp_ins = []
            for t in range(NT):
                dump = dp.tile([128, C], F32, tag="dump")
                rsum = smp.tile([128, 1], F32, tag="rsum")
                e = nc.scalar.activation(out=dump, in_=xloc[t], func=AF.Exp,
                                         bias=shA, scale=1.0, accum_out=rsum)
                exp_ins.append(e)
                nc.vector.reciprocal(out=negrr[:, t:t + 1], in_=rsum)
                nc.vector.tensor_scalar(out=negrr[:, t:t + 1],
                                        in0=negrr[:, t:t + 1], scalar1=-1.0,
                                        scalar2=None, op0=OP.mult)
                ht = pp.tile([128, C], BF16, tag=f"hb{t}")
                nc.vector.scalar_tensor_tensor(out=ht, in0=dump,
                                               scalar=negrr[:, t:t + 1],
                                               in1=eqm[t], op0=OP.mult,
                                               op1=OP.subtract)
                hb.append(ht)

            # xp gather (host-precomputed pos indices): per-tile issues.
            # Dep-pinned after tile 3's exp so the gather DMA does not
            # steal HBM bandwidth from the xloc load.
            xpg = []
            for t in range(NT):
                xpt = gxp.tile([128, C], F32, tag="xp")
                gi = nc.gpsimd.indirect_dma_start(
                    out=xpt, out_offset=None,
                    in_=x_d.ap(),
                    in_offset=bass.IndirectOffsetOnAxis(
                        ap=posloc[:, t:t + 1], axis=0))
                add_dep_helper(gi.ins, exp_ins[0].ins, sync=False)
                xpg.append(xpt)

            # transpose to [class, row]; top-1 + index per class
            ccall = pp.tile([128, 2 * CT], F32, tag="ccall")
            idxcat = pp.tile([128, CT], F32, tag="idxcat")
            for g in range(4):
                psts = []
                for ci in range(2):
                    pst = psa.tile([128, C], BF16, tag=f"pst{ci}")
                    psts.append(pst)
                for t in range(NT):
                    for ci in range(2):
                        ct = g * 2 + ci
                        nc.tensor.transpose(
                            out=psts[ci][:, t * 128:(t + 1) * 128],
                            in_=hb[t][:, ct * 128:(ct + 1) * 128],
                            identity=identb)
                for ci in range(2):
                    ct = g * 2 + ci
                    nvT = nvp.tile([128, C], BF16, tag="nvT")
                    if ci == 0:
                        nc.scalar.copy(out=nvT, in_=psts[ci])
                    else:
                        nc.vector.tensor_copy(out=nvT, in_=psts[ci])
                    top8v = smp.tile([128, 8], BF16, tag="top8v")
                    nc.vector.max(out=top8v, in_=nvT)
                    top8i = smp.tile([128, 8], U32, tag="top8i")
                    nc.vector.max_index(out=top8i, in_max=top8v, in_values=nvT)
                    nc.vector.tensor_copy(out=idxcat[:, ct:ct + 1],
                                          in_=top8i[:, 0:1])
                    nc.vector.tensor_copy(out=ccall[:, ct:ct + 1],
                                          in_=top8v[:, 0:1])
            # enc = (BIGI - core_off) - idx, all classes in one op
            nc.vector.tensor_scalar(out=ccall[:, CT:2 * CT], in0=idxcat,
                                    scalar1=bigoff, scalar2=-1.0,
                                    op0=OP.subtract, op1=OP.mult)
            nc.sync.dma_start(out=cc_in.ap(), in_=ccall)

            # ---------- AllGather ----------
            nc.gpsimd.collective_compute(
                "AllGather", OP.bypass,
                replica_groups=[list(range(NCORES))],
                ins=[cc_in.ap().opt()], outs=[cc_out.ap().opt()])

            g8 = pp.tile([128, NCORES, 2 * CT], F32, tag="g8")
            gsrc = bass.AP(tensor=cc_out.ap().tensor, offset=0,
                           ap=[[2 * CT, 128], [128 * 2 * CT, NCORES],
                               [1, 2 * CT]])
            nc.scalar.dma_start(out=g8, in_=gsrc)

            # ---- overlapped with the collective: fold xp rows into xloc ----
            for t in range(NT):
                nc.vector.tensor_tensor(out=xloc[t], in0=xloc[t],
                                        in1=xpg[t], op=OP.add)

            # ---------- global combine: per-class hardest negative ----------
            negidxf = pp.tile([128, CT], F32, tag="negidxf")
            for ct in range(CT):
                gv = smp.tile([128, 1], F32, tag="gv")
                nc.vector.tensor_reduce(out=gv, in_=g8[:, :, ct], axis=AX.X,
                                        op=OP.max)
                mm = smp.tile([128, NCORES], F32, tag="mm")
                nc.vector.tensor_tensor(out=mm, in0=g8[:, :, ct],
                                        in1=gv.to_broadcast([128, NCORES]),
                                        op=OP.is_ge)
                cand = smp.tile([128, NCORES], F32, tag="cand")
                nc.vector.tensor_tensor(out=cand, in0=mm,
                                        in1=g8[:, :, CT + ct], op=OP.mult)
                genc = smp.tile([128, 1], F32, tag="genc")
                nc.vector.tensor_reduce(out=genc, in_=cand, axis=AX.X,
                                        op=OP.max)
                nc.vector.tensor_scalar(out=negidxf[:, ct:ct + 1], in0=genc,
                                        scalar1=-1.0, scalar2=BIGI,
                                        op0=OP.mult, op1=OP.add)
            negtabi = pp.tile([128, CT], I32, tag="negtabi")
            nc.vector.tensor_copy(out=negtabi, in_=negidxf)
            ntdst = bass.AP(tensor=negtab_d.ap().tensor, offset=0,
                            ap=[[1, 128], [128, CT]])
            nc.sync.dma_start(out=ntdst, in_=negtabi)

            # ---------- phase C: per-anchor neg (double hop) + CE ----------
            ext = pp.tile([128, NT], F32, tag="ext")
            rsumC = pp.tile([128, NT], F32, tag="rsumC")
            lnrC = pp.tile([128, NT], F32, tag="lnrC")
            expC_ins = []
            for t in range(NT):
                negoff = smp.tile([128, 1], I32, tag="negoff")
                nc.gpsimd.indirect_dma_start(
                    out=negoff, out_offset=None,
                    in_=negtab_d.ap(),
                    in_offset=bass.IndirectOffsetOnAxis(
                        ap=tgtloc[:, t:t + 1], axis=0))
                xnt = gxn.tile([128, C], F32, tag="xn")
                nc.gpsimd.indirect_dma_start(
                    out=xnt, out_offset=None,
                    in_=x_d.ap(),
                    in_offset=bass.IndirectOffsetOnAxis(ap=negoff, axis=0))
                nc.vector.tensor_tensor(out=xloc[t], in0=xloc[t], in1=xnt,
                                        op=OP.add)
                dumpE = dp.tile([128, C], F32, tag="dump")
                nc.vector.tensor_tensor(out=dumpE, in0=xloc[t], in1=eqm[t],
                                        op=OP.mult)
                nc.vector.tensor_reduce(out=ext[:, t:t + 1], in_=dumpE,
                                        axis=AX.X, op=OP.add)
                dump = dp.tile([128, C], F32, tag="dump")
                rsc = smp.tile([128, 1], F32, tag="rsc")
                e = nc.scalar.activation(out=dump, in_=xloc[t], func=AF.Exp,
                                         bias=shC, scale=1.0, accum_out=rsc)
                nc.vector.tensor_copy(out=rsumC[:, t:t + 1], in_=rsc)
                expC_ins.append(e)
            ln = nc.scalar.activation(out=lnrC, in_=rsumC, func=AF.Ln)
            add_dep_helper(ln.ins, expC_ins[-1].ins, sync=False)

            # loss partial: sum_t [ln(rsum) + SHIFT_C - tval]
            li = pp.tile([128, NT], F32, tag="li")
            nc.vector.tensor_scalar(out=li, in0=lnrC, scalar1=SHIFT_C,
                                    scalar2=None, op0=OP.add)
            nc.vector.tensor_tensor(out=li, in0=li, in1=ext, op=OP.subtract)
            acc = pp.tile([128, 1], F32, tag="acc")
            nc.vector.tensor_reduce(out=acc, in_=li, axis=AX.X, op=OP.add)

            pss = psb.tile([1, 1], F32, tag="psum_out")
            nc.tensor.matmul(pss, lhsT=acc, rhs=ones, start=True, stop=True)
            outt = smp.tile([1, 1], F32, tag="outt")
            nc.vector.tensor_copy(out=outt, in_=pss)
            nc.sync.dma_start(out=out_d.ap(), in_=outt)

    nc.compile()
    return nc


_NC_CACHE = {}


def get_nc():
    if "nc" not in _NC_CACHE:
        _NC_CACHE["nc"] = build_nc()
    return _NC_CACHE["nc"]


def _pos_idx_host(tgt):
    """pos_idx[i] = first same-label index != i (argmax semantics of the
    reference: first j with target[j]==target[i], j != i)."""
    order = np.argsort(tgt, kind="stable")
    pos = np.zeros(B, dtype=np.int32)
    srt = tgt[order]
    starts = np.searchsorted(srt, np.arange(srt[-1] + 1), side="left")
    for c in range(len(starts)):
        lo = starts[c]
        hi = starts[c + 1] if c + 1 < len(starts) else B
        idxs = order[lo:hi]
        if len(idxs) == 0:
            continue
        mn = idxs.min()
        pos[idxs] = mn
        if len(idxs) > 1:
            second = np.partition(idxs, 1)[1]
            pos[mn] = second
        else:
            pos[mn] = 0
    return pos


def make_in_maps(x, target):
    x = np.ascontiguousarray(np.asarray(x, dtype=np.float32))
    tgt = np.asarray(target).astype(np.int64)
    assert x.shape == (B, C) and tgt.shape == (B,)

    import ml_dtypes
    cid = np.arange(C, dtype=np.float32)
    cidb_full = np.ascontiguousarray(np.broadcast_to(cid, (128, C)))
    identb = np.eye(128, dtype=ml_dtypes.bfloat16)

    pos = _pos_idx_host(tgt)

    in_maps = []
    for k in range(NCORES):
        rows = slice(k * BL, (k + 1) * BL)
        tl = tgt[rows].astype(np.float32)
        in_maps.append({
            "x": x,
            "xloc": np.ascontiguousarray(x[rows]),
            "cidb": cidb_full,
            "tcols": np.ascontiguousarray(tl.reshape(NT, 128).T),
            "posloc": np.ascontiguousarray(
                pos[rows].reshape(NT, 128).T.astype(np.int32)),
            "tgtloc": np.ascontiguousarray(
                tgt[rows].reshape(NT, 128).T.astype(np.int32)),
            "bigoff": np.full((128, 1), BIGI - k * BL, dtype=np.float32),
            "identb": identb,
        })
    return in_maps


def kernel(x, target):
    nc = get_nc()
    in_maps = make_in_maps(x, target)
    res = run_bass_kernel_spmd(nc, in_maps, core_ids=list(range(NCORES)))
    total = sum(float(res.results[k]["partial"][0, 0]) for k in range(NCORES))
    return np.float32(total / B)


# revision 30
# speedup vs baseline: 1.8991x; 1.1425x over previous
"""Trainium2 Bass kernel for the hard-negative-mining set loss (v6).

Structure (all HW-probe-proven primitives):
  * host precomputes pos_idx (pure target metadata); phase B eliminated.
  * mining (per core, 1024 local rows): exp pass -> rsum; DVE computes
    bf16 mining values -probs - onehot; PE-transposes (bf16) to
    [class, row]; DVE max8 + first-index per class into 3D top8 tiles;
    8KB AllGather of per-class (vmax, enc) tables.
  * combine -> per-class neg row table in DRAM -> ONE batched
    single-element indirect gather by target -> per-anchor neg index.
  * xp/xn rows: plain per-tile indirect gathers + DVE adds; one fused
    target-logit extraction per tile on the summed logits.
HW notes (probed): tensor_tensor_reduce, batched multi-row gathers,
batched-offset + compute_op=add chains all hang/corrupt on HW; batched
single-element gathers and the DRAM-table double hop work.
"""

import numpy as np

import concourse.bass as bass
import concourse.bacc as bacc
import concourse.tile as tile
from concourse import mybir
from concourse.bass_utils import run_bass_kernel_spmd
from concourse.tile import add_dep_helper

B, C = 8192, 1024
NCORES = 8
BL = B // NCORES      # 1024 local rows per core
NT = BL // 128        # 8 row tiles
CT = C // 128         # 8 class tiles
BIGI = 16384.0        # index encoding base: enc = BIGI - global_row_idx
SHIFT_A = 10.0        # softmax shift (x ~ N(0,1): rowmax << SHIFT_A)
SHIFT_C = 14.0        # summed-logits shift (3 logits per entry)
F32 = mybir.dt.float32
BF16 = mybir.dt.bfloat16
I32 = mybir.dt.int32
U32 = mybir.dt.uint32
AX = mybir.AxisListType
OP = mybir.AluOpType
AF = mybir.ActivationFunctionType


def build_nc():
    nc = bacc.Bacc("TRN2", target_bir_lowering=False, debug=False,
                   num_devices=NCORES)

    x_d = nc.dram_tensor("x", [B, C], F32, kind="ExternalInput")
    xloc_d = nc.dram_tensor("xloc", [BL, C], F32, kind="ExternalInput")
    cidb_d = nc.dram_tensor("cidb", [128, C], F32, kind="ExternalInput")
    tcols_d = nc.dram_tensor("tcols", [128, NT], F32, kind="ExternalInput")
    posloc_d = nc.dram_tensor("posloc", [128, NT], I32, kind="ExternalInput")
    tgtloc_d = nc.dram_tensor("tgtloc", [128, NT], I32, kind="ExternalInput")
    bigoff_d = nc.dram_tensor("bigoff", [128, 1], F32, kind="ExternalInput")
    identb_d = nc.dram_tensor("identb", [128, 128], BF16, kind="ExternalInput")
    out_d = nc.dram_tensor("partial", [1, 1], F32, kind="ExternalOutput")

    # collective bounce: col ct = vmax of class ct*128+p, col CT+ct = enc
    cc_in = nc.dram_tensor("cc_in", [128, 2 * CT], F32)
    cc_out = nc.dram_tensor("cc_out", [NCORES, 128, 2 * CT], F32)
    negtab_d = nc.dram_tensor("negtab", [C, 1], I32)

    with tile.TileContext(nc) as tc:
        with (
            tc.tile_pool(name="persist", bufs=1) as pp,
            tc.tile_pool(name="dumppool", bufs=2) as dp,
            tc.tile_pool(name="nvpool", bufs=2) as nvp,
            tc.tile_pool(name="small", bufs=4) as smp,
            tc.tile_pool(name="gxp", bufs=4) as gxp,
            tc.tile_pool(name="gxn", bufs=4) as gxn,
            tc.tile_pool(name="psA", bufs=2, space="PSUM") as psa,
            tc.tile_pool(name="psB", bufs=2, space="PSUM") as psb,
        ):
            # ---------- input loads ----------
            cidb = pp.tile([128, C], F32, tag="cidb")
            nc.scalar.dma_start(out=cidb, in_=cidb_d.ap())
            tcols = pp.tile([128, NT], F32, tag="tcols")
            nc.scalar.dma_start(out=tcols, in_=tcols_d.ap())
            posloc = pp.tile([128, NT], I32, tag="posloc")
            nc.scalar.dma_start(out=posloc, in_=posloc_d.ap())
            tgtloc = pp.tile([128, NT], I32, tag="tgtloc")
            nc.scalar.dma_start(out=tgtloc, in_=tgtloc_d.ap())
            bigoff = pp.tile([128, 1], F32, tag="bigoff")
            nc.scalar.dma_start(out=bigoff, in_=bigoff_d.ap())
            identb = pp.tile([128, 128], BF16, tag="identb")
            nc.scalar.dma_start(out=identb, in_=identb_d.ap())

            # xloc: 8 tiles, round-robin across the two hwdge issue queues
            dma_engs = [nc.sync, nc.scalar]
            xloc = []
            for t in range(NT):
                xt = pp.tile([128, C], F32, tag=f"xloc{t}")
                dma_engs[t % 2].dma_start(
                    out=xt, in_=xloc_d.ap()[t * 128:(t + 1) * 128, :])
                xloc.append(xt)

            ones = pp.tile([128, 1], F32, tag="ones")
            nc.vector.memset(ones, 1.0)
            shA = pp.tile([128, 1], F32, tag="shA")
            nc.vector.memset(shA, -SHIFT_A)
            shC = pp.tile([128, 1], F32, tag="shC")
            nc.vector.memset(shC, -SHIFT_C)

            # ---------- phase A: hardest-negative mining (exp domain, bf16) --
            eqm = []      # onehot(target), f32: mining exclusion + extraction
            for t in range(NT):
                eq = pp.tile([128, C], F32, tag=f"eqm{t}")
                nc.vector.tensor_scalar(out=eq, in0=cidb,
                                        scalar1=tcols[:, t:t + 1], scalar2=None,
                                        op0=OP.is_equal)
                eqm.append(eq)

            negrr = pp.tile([128, NT], F32, tag="negrr")
            hb = []       # mining values, bf16: -probs - onehot
            exp_ins = []
            for t in range(NT):
                dump = dp.tile([128, C], F32, tag="dump")
                rsum = smp.tile([128, 1], F32, tag="rsum")
                e = nc.scalar.activation(out=dump, in_=xloc[t], func=AF.Exp,
                                         bias=shA, scale=1.0, accum_out=rsum)
                exp_ins.append(e)
                nc.vector.reciprocal(out=negrr[:, t:t + 1], in_=rsum)
                nc.vector.tensor_scalar(out=negrr[:, t:t + 1],
                                        in0=negrr[:, t:t + 1], scalar1=-1.0,
                                        scalar2=None, op0=OP.mult)
                ht = pp.tile([128, C], BF16, tag=f"hb{t}")
                nc.vector.scalar_tensor_tensor(out=ht, in0=dump,
                                               scalar=negrr[:, t:t + 1],
                                               in1=eqm[t], op0=OP.mult,
                                               op1=OP.subtract)
                hb.append(ht)

            # xp gather (host-precomputed pos indices): per-tile issues.
            # Dep-pinned after tile 3's exp so the gather DMA does not
            # steal HBM bandwidth from the xloc load.
            xpg = []
            for t in range(NT):
                xpt = gxp.tile([128, C], F32, tag="xp")
                gi = nc.gpsimd.indirect_dma_start(
                    out=xpt, out_offset=None,
                    in_=x_d.ap(),
                    in_offset=bass.IndirectOffsetOnAxis(
                        ap=posloc[:, t:t + 1], axis=0))
                add_dep_helper(gi.ins, exp_ins[0].ins, sync=False)
                xpg.append(xpt)

            # transpose to [class, row]; top-1 + index per class
            ccall = pp.tile([128, 2 * CT], F32, tag="ccall")
            idxcat = pp.tile([128, CT], F32, tag="idxcat")
            for g in range(4):
                psts = []
                for ci in range(2):
                    pst = psa.tile([128, C], BF16, tag=f"pst{ci}")
                    psts.append(pst)
                for t in range(NT):
                    for ci in range(2):
                        ct = g * 2 + ci
                        nc.tensor.transpose(
                            out=psts[ci][:, t * 128:(t + 1) * 128],
                            in_=hb[t][:, ct * 128:(ct + 1) * 128],
                            identity=identb)
                for ci in range(2):
                    ct = g * 2 + ci
                    nvT = nvp.tile([128, C], BF16, tag="nvT")
                    if ci == 0:
                        nc.scalar.copy(out=nvT, in_=psts[ci])
                    else:
                        nc.vector.tensor_copy(out=nvT, in_=psts[ci])
                    top8v = smp.tile([128, 8], BF16, tag="top8v")
                    nc.vector.max(out=top8v, in_=nvT)
                    top8i = smp.tile([128, 8], U32, tag="top8i")
                    nc.vector.max_index(out=top8i, in_max=top8v, in_values=nvT)
                    nc.vector.tensor_copy(out=idxcat[:, ct:ct + 1],
                                          in_=top8i[:, 0:1])
                    nc.vector.tensor_copy(out=ccall[:, ct:ct + 1],
                                          in_=top8v[:, 0:1])
            # enc = (BIGI - core_off) - idx, all classes in one op
            nc.vector.tensor_scalar(out=ccall[:, CT:2 * CT], in0=idxcat,
                                    scalar1=bigoff, scalar2=-1.0,
                                    op0=OP.subtract, op1=OP.mult)
            nc.sync.dma_start(out=cc_in.ap(), in_=ccall)

            # ---------- AllGather ----------
            nc.gpsimd.collective_compute(
                "AllGather", OP.bypass,
                replica_groups=[list(range(NCORES))],
                ins=[cc_in.ap().opt()], outs=[cc_out.ap().opt()])

            g8 = pp.tile([128, NCORES, 2 * CT], F32, tag="g8")
            gsrc = bass.AP(tensor=cc_out.ap().tensor, offset=0,
                           ap=[[2 * CT, 128], [128 * 2 * CT, NCORES],
                               [1, 2 * CT]])
            nc.scalar.dma_start(out=g8, in_=gsrc)

            # ---- overlapped with the collective: fold xp rows into xloc ----
            for t in range(NT):
                nc.vector.tensor_tensor(out=xloc[t], in0=xloc[t],
                                        in1=xpg[t], op=OP.add)

            # ---------- global combine: per-class hardest negative ----------
            negidxf = pp.tile([128, CT], F32, tag="negidxf")
            for ct in range(CT):
                gv = smp.tile([128, 1], F32, tag="gv")
                nc.vector.tensor_reduce(out=gv, in_=g8[:, :, ct], axis=AX.X,
                                        op=OP.max)
                mm = smp.tile([128, NCORES], F32, tag="mm")
                nc.vector.tensor_tensor(out=mm, in0=g8[:, :, ct],
                                        in1=gv.to_broadcast([128, NCORES]),
                                        op=OP.is_ge)
                cand = smp.tile([128, NCORES], F32, tag="cand")
                nc.vector.tensor_tensor(out=cand, in0=mm,
                                        in1=g8[:, :, CT + ct], op=OP.mult)
                genc = smp.tile([128, 1], F32, tag="genc")
                nc.vector.tensor_reduce(out=genc, in_=cand, axis=AX.X,
                                        op=OP.max)
                nc.vector.tensor_scalar(out=negidxf[:, ct:ct + 1], in0=genc,
                                        scalar1=-1.0, scalar2=BIGI,
                                        op0=OP.mult, op1=OP.add)
            negtabi = pp.tile([128, CT], I32, tag="negtabi")
            nc.vector.tensor_copy(out=negtabi, in_=negidxf)
            ntdst = bass.AP(tensor=negtab_d.ap().tensor, offset=0,
                            ap=[[1, 128], [128, CT]])
            # issue on the gpsimd queue: the negoff gathers right after it
            # stay in-queue (no cross-engine semaphore round trip)
            nc.gpsimd.dma_start(out=ntdst, in_=negtabi)

            # per-anchor neg row index: all 8 table gathers issued up front
            # so their DMA round-trips overlap (each [128,1]-offset gather
            # lands in its own column of one persistent tile)
            negoff = pp.tile([128, NT], I32, tag="negoff")
            for t in range(NT):
                nc.gpsimd.indirect_dma_start(
                    out=negoff[:, t:t + 1], out_offset=None,
                    in_=negtab_d.ap(),
                    in_offset=bass.IndirectOffsetOnAxis(
                        ap=tgtloc[:, t:t + 1], axis=0))

            # ---------- phase C: xn gather + CE over summed logits ----------
            ext = pp.tile([128, NT], F32, tag="ext")
            rsumC = pp.tile([128, NT], F32, tag="rsumC")
            lnrC = pp.tile([128, NT], F32, tag="lnrC")
            expC_ins = []
            for t in range(NT):
                xnt = gxn.tile([128, C], F32, tag="xn")
                nc.gpsimd.indirect_dma_start(
                    out=xnt, out_offset=None,
                    in_=x_d.ap(),
                    in_offset=bass.IndirectOffsetOnAxis(
                        ap=negoff[:, t:t + 1], axis=0))
                nc.vector.tensor_tensor(out=xloc[t], in0=xloc[t], in1=xnt,
                                        op=OP.add)
                dumpE = dp.tile([128, C], F32, tag="dump")
                nc.vector.tensor_tensor(out=dumpE, in0=xloc[t], in1=eqm[t],
                                        op=OP.mult)
                nc.vector.tensor_reduce(out=ext[:, t:t + 1], in_=dumpE,
                                        axis=AX.X, op=OP.add)
                dump = dp.tile([128, C], F32, tag="dump")
                rsc = smp.tile([128, 1], F32, tag="rsc")
                e = nc.scalar.activation(out=dump, in_=xloc[t], func=AF.Exp,
                                         bias=shC, scale=1.0, accum_out=rsc)
                nc.vector.tensor_copy(out=rsumC[:, t:t + 1], in_=rsc)
                expC_ins.append(e)
            ln = nc.scalar.activation(out=lnrC, in_=rsumC, func=AF.Ln)
            add_dep_helper(ln.ins, expC_ins[-1].ins, sync=False)

            # loss partial: sum_t [ln(rsum) + SHIFT_C - tval]
            li = pp.tile([128, NT], F32, tag="li")
            nc.vector.tensor_scalar(out=li, in0=lnrC, scalar1=SHIFT_C,
                                    scalar2=None, op0=OP.add)
            nc.vector.tensor_tensor(out=li, in0=li, in1=ext, op=OP.subtract)
            acc = pp.tile([128, 1], F32, tag="acc")
            nc.vector.tensor_reduce(out=acc, in_=li, axis=AX.X, op=OP.add)

            pss = psb.tile([1, 1], F32, tag="psum_out")
            nc.tensor.matmul(pss, lhsT=acc, rhs=ones, start=True, stop=True)
            outt = smp.tile([1, 1], F32, tag="outt")
            nc.vector.tensor_copy(out=outt, in_=pss)
            nc.sync.dma_start(out=out_d.ap(), in_=outt)

    nc.compile()
    return nc


_NC_CACHE = {}


def get_nc():
    if "nc" not in _NC_CACHE:
        _NC_CACHE["nc"] = build_nc()
    return _NC_CACHE["nc"]


def _pos_idx_host(tgt):
    """pos_idx[i] = first same-label index != i (argmax semantics of the
    reference: first j with target[j]==target[i], j != i)."""
    order = np.argsort(tgt, kind="stable")
    pos = np.zeros(B, dtype=np.int32)
    srt = tgt[order]
    starts = np.searchsorted(srt, np.arange(srt[-1] + 1), side="left")
    for c in range(len(starts)):
        lo = starts[c]
        hi = starts[c + 1] if c + 1 < len(starts) else B
        idxs = order[lo:hi]
        if len(idxs) == 0:
            continue
        mn = idxs.min()
        pos[idxs] = mn
        if len(idxs) > 1:
            second = np.partition(idxs, 1)[1]
            pos[mn] = second
        else:
            pos[mn] = 0
    return pos


def make_in_maps(x, target):
    x = np.ascontiguousarray(np.asarray(x, dtype=np.float32))
    tgt = np.asarray(target).astype(np.int64)
    assert x.shape == (B, C) and tgt.shape == (B,)

    import ml_dtypes
    cid = np.arange(C, dtype=np.float32)
    cidb_full = np.ascontiguousarray(np.broadcast_to(cid, (128, C)))
    identb = np.eye(128, dtype=ml_dtypes.bfloat16)

    pos = _pos_idx_host(tgt)

    in_maps = []
    for k in range(NCORES):
        rows = slice(k * BL, (k + 1) * BL)
        tl = tgt[rows].astype(np.float32)
        in_maps.append({
            "x": x,
            "xloc": np.ascontiguousarray(x[rows]),
            "cidb": cidb_full,
            "tcols": np.ascontiguousarray(tl.reshape(NT, 128).T),
            "posloc": np.ascontiguousarray(
                pos[rows].reshape(NT, 128).T.astype(np.int32)),
            "tgtloc": np.ascontiguousarray(
                tgt[rows].reshape(NT, 128).T.astype(np.int32)),
            "bigoff": np.full((128, 1), BIGI - k * BL, dtype=np.float32),
            "identb": identb,
        })
    return in_maps


def kernel(x, target):
    nc = get_nc()
    in_maps = make_in_maps(x, target)
    res = run_bass_kernel_spmd(nc, in_maps, core_ids=list(range(NCORES)))
    total = sum(float(res.results[k]["partial"][0, 0]) for k in range(NCORES))
    return np.float32(total / B)
